# revision 16
# baseline (speedup 1.0000x reference)
"""Trainium2 Bass kernel for a neural-cellular-automata step.

out = clip(x + MLP(depthwise3x3(x)), 0, 1) on [8, 512, 512, 16] fp32, NHWC.

Strategy: data-parallel over batch (1 image per NeuronCore, 8 cores).
x is pre-cast to bf16 on the host (tolerance is 2e-2; bf16 keeps us near
4e-3) which halves DMA traffic and makes every PE op run at 1 cycle/row.

Per core the image is processed in 86 column groups: each group's tile
covers 8 w-columns (128 partitions = 8w x 16c) but only the 6 interior
columns are produced, so the 3x3 depthwise taps never cross tiles and no
edge-correction matmuls are needed (tiles overlap by 2 w; group stride 6).
A PE transpose (vs a bf16 identity) turns the native [rows, (w,c)] layout
into X_T [(w,c), rows]; the depthwise perception conv fused with W1
becomes 3 row-shifted bf16 matmuls with block-structured stationary
operands built host-side.  ReLU+bias runs on the scalar engine (bf16
out), the second 1x1 conv uses h as the stationary operand so dx lands
directly back in native layout in PSUM, accumulated over 30-w-column
spans.  The vector engine applies x + dx and the [0,1] clamp into a bf16
staging tile that is DMA-stored incrementally; input DMA is chunked so
compute starts after ~3% of the load.
"""

import contextlib

import numpy as np
import ml_dtypes

import concourse.mybir as mybir
import concourse.tile as tile
from concourse import bacc
from concourse.bass_utils import run_bass_kernel_spmd

B, H, W, C, K, HID = 8, 512, 512, 16, 3, 128
F32 = mybir.dt.float32
F32R = mybir.dt.float32r
BF16 = mybir.dt.bfloat16
GW = 6  # output columns per group (tile covers 8, halo 1 each side)
NG = 86  # 85 full groups + tail group with 2 outputs
PAD_L = 16  # one zero w-column (16 ch) left pad, in elements
ABW = PAD_L + W * C + 80  # band width; right pad so group 85's tile fits
SPAN_W = 30  # w-columns per PSUM dx span (480 native cols)
NBAND = H // 128  # 4 row bands

_CACHE = {}


def _default_perc():
    ident = np.array([[0, 0, 0], [0, 1, 0], [0, 0, 0]], np.float32)
    sx = np.array([[-1, 0, 1], [-2, 0, 2], [-1, 0, 1]], np.float32) / 8.0
    sy = sx.T
    return np.stack([ident, sx, sy], axis=-1)  # [3,3,K]


def _build_lm(perc, W1):
    """Stationary operands for the fused perception+W1 matmuls.

    Block t = (wsl-1)*3 + dy is [128, HID]: rows [wl*16, wl*16+16) hold
    T[dy, 1+(wl-wsl)] for |wl-wsl| <= 1, where wsl in 1..6 is the output
    column within the 8-wide tile and wl the tile-local input column.
    """
    Wk = W1.reshape(C, K, HID)
    T = np.einsum("yxk,ckd->yxcd", perc, Wk).astype(np.float32)  # [3,3,C,HID]
    LM = np.zeros((6, 3, 128, HID), np.float32)
    for wsl in range(1, 7):
        for dy in range(3):
            for dxx in (-1, 0, 1):
                wl = wsl + dxx
                LM[wsl - 1, dy, wl * 16 : wl * 16 + 16, :] = T[dy, 1 + dxx]
    lm = np.ascontiguousarray(LM.transpose(2, 0, 1, 3).reshape(128, 18 * HID))
    return lm.astype(ml_dtypes.bfloat16)


def _build_module():
    nc = bacc.Bacc("TRN2", target_bir_lowering=False)
    XBF = nc.dram_tensor("XBF", [H, W * C], BF16, kind="ExternalInput")
    LM = nc.dram_tensor("LM", [128, 18 * HID], BF16, kind="ExternalInput")
    W2T = nc.dram_tensor("W2T", [HID, C], BF16, kind="ExternalInput")
    B1 = nc.dram_tensor("B1", [HID, 1], F32, kind="ExternalInput")
    IDENT = nc.dram_tensor("IDENT", [128, 128], BF16, kind="ExternalInput")
    OUTBF = nc.dram_tensor("OUTBF", [H, W * C], BF16, kind="ExternalOutput")

    with tile.TileContext(nc) as tc, contextlib.ExitStack() as ctx:
        consts = ctx.enter_context(tc.tile_pool(name="consts", bufs=1))
        hpool = ctx.enter_context(tc.tile_pool(name="hsb", bufs=14))
        otpool = ctx.enter_context(tc.tile_pool(name="ot", bufs=8))
        ps_tr = ctx.enter_context(tc.tile_pool(name="ps_tr", bufs=1, space="PSUM"))
        ps_h = ctx.enter_context(tc.tile_pool(name="ps_h", bufs=5, space="PSUM"))
        ps_dxt = ctx.enter_context(tc.tile_pool(name="ps_dxt", bufs=1, space="PSUM"))

        lm = consts.tile([128, 18 * HID], BF16)
        nc.sync.dma_start(lm, LM[:])
        w2 = consts.tile([HID, C], BF16)
        nc.sync.dma_start(w2, W2T[:])
        b1 = consts.tile([HID, 1], F32)
        nc.sync.dma_start(b1, B1[:])
        ident = consts.tile([128, 128], BF16)
        nc.sync.dma_start(ident, IDENT[:])

        ab = []
        for b in range(NBAND):
            t = consts.tile([128, ABW], BF16, tag=f"a{b}", name=f"a{b}")
            nc.gpsimd.memset(t[:, 0:PAD_L], 0.0)
            nc.gpsimd.memset(t[:, PAD_L + W * C :], 0.0)
            ab.append(t)
        # chunked loads, interleaved across bands so group 0 unblocks early
        for c in range(8):
            for b in range(NBAND):
                nc.sync.dma_start(
                    ab[b][:, PAD_L + 1024 * c : PAD_L + 1024 * (c + 1)],
                    XBF[:][128 * b : 128 * b + 128, 1024 * c : 1024 * (c + 1)],
                )

        xts = []
        for i in range(2):
            t = consts.tile([128, 514], BF16, tag=f"xt{i}", name=f"xt{i}")
            nc.gpsimd.memset(t[:, 0:1], 0.0)
            nc.gpsimd.memset(t[:, 513:514], 0.0)
            xts.append(t)

        def lm_ap(wsl, dy):
            t = (wsl - 1) * 3 + dy
            return lm[:, t * HID : (t + 1) * HID]

        def build_xt(j):
            """PE-transpose the 8-w-column window of group j into xts[j%2]."""
            xt = xts[j % 2]
            ptr = ps_tr.tile([128, 512], BF16, tag="tr")
            for b in range(NBAND):
                nc.tensor.matmul(
                    ptr[:, 128 * b : 128 * b + 128],
                    ab[b][:, 96 * j : 96 * j + 128],
                    ident,
                    start=True,
                    stop=True,
                    is_transpose=True,
                )
            nc.vector.tensor_copy(xt[:, 1:513], ptr[:])

        build_xt(0)
        # dx spans: 2 groups (12 w) x 2 bands share one PSUM bank:
        # dxt[p] holds bands 2p, 2p+1 at column offset (b%2)*192.
        dxt = None
        pending = None  # (j, h-pairs) whose W2 matmuls are deferred one group

        def emit_w2(jj, hps):
            nonlocal dxt
            if jj % 2 == 0:
                dxt = [
                    ps_dxt.tile([128, 384], F32, tag=f"d{p}", name=f"d{p}")
                    for p in range(2)
                ]
            nout = 2 if jj == NG - 1 else GW
            for wsl in range(1, nout + 1):
                w = 6 * jj - 1 + wsl
                off = (w % 12) * 16
                for b in range(NBAND):
                    nc.tensor.matmul(
                        dxt[b // 2][:, (b % 2) * 192 + off : (b % 2) * 192 + off + 16],
                        hps[wsl - 1][:, 128 * b : 128 * b + 128],
                        w2[:],
                        start=True,
                        stop=True,
                    )

        def emit_add_store(jj):
            """Residual add + clamp + store once group jj completed a span."""
            if jj % 2 != 1 and jj != NG - 1:
                return
            S = jj // 2
            ncb = 192 if S < 42 else 128
            for b in range(NBAND):
                ot = otpool.tile([128, ncb], BF16, tag="ot")
                nc.vector.tensor_tensor(
                    ot,
                    dxt[b // 2][:, (b % 2) * 192 : (b % 2) * 192 + ncb],
                    ab[b][:, PAD_L + 192 * S : PAD_L + 192 * S + ncb],
                    op=mybir.AluOpType.add,
                )
                nc.vector.tensor_scalar(
                    ot, ot, 0.0, 1.0, mybir.AluOpType.max, mybir.AluOpType.min
                )
                nc.sync.dma_start(
                    OUTBF[:][128 * b : 128 * b + 128, 192 * S : 192 * S + ncb],
                    ot,
                )

        for j in range(NG):
            if j + 1 < NG:
                build_xt(j + 1)
            xt = xts[j % 2]

            nout = 2 if j == NG - 1 else GW
            hps = []
            for wsl in range(1, nout + 1):
                hp = ps_h.tile([128, 512], F32, tag="h")
                for dy in range(3):
                    nc.tensor.matmul(
                        hp,
                        lm_ap(wsl, dy),
                        xt[:, dy : dy + 512],
                        start=(dy == 0),
                        stop=(dy == 2),
                    )
                h = hpool.tile([HID, 512], BF16, tag="h")
                nc.scalar.activation(
                    h, hp, mybir.ActivationFunctionType.Relu, bias=b1, scale=1.0
                )
                hps.append(h)

            if pending is not None:
                jj, phps = pending
                emit_w2(jj, phps)
                emit_add_store(jj)
            pending = (j, hps)

        jj, phps = pending
        emit_w2(jj, phps)
        emit_add_store(jj)

    nc.finalize()
    return nc


def kernel(x, perc=None, W1=None, b1=None, W2=None, b2=None, lock_release=None, **_):
    x = np.asarray(x)
    assert x.shape == (B, H, W, C)
    x = np.ascontiguousarray(x, np.float32)
    perc = _default_perc() if perc is None else np.asarray(perc, np.float32)
    W1 = np.asarray(W1, np.float32)
    W2 = np.asarray(W2, np.float32)
    b1 = np.zeros(HID, np.float32) if b1 is None else np.asarray(b1, np.float32)
    b2 = np.zeros(C, np.float32) if b2 is None else np.asarray(b2, np.float32)
    assert not np.any(b2 != 0.0), "fast path assumes b2 == 0"

    key = ("mod", False)
    if key not in _CACHE:
        _CACHE[key] = _build_module()
    nc = _CACHE[key]

    base_map = {
        "LM": _build_lm(perc, W1),
        "W2T": np.ascontiguousarray(W2).astype(ml_dtypes.bfloat16),
        "B1": np.ascontiguousarray(b1[:, None]),
        "IDENT": np.eye(128, dtype=np.float32).astype(ml_dtypes.bfloat16),
    }
    xbf = x.reshape(B, H, W * C).astype(ml_dtypes.bfloat16)

    in_maps = []
    for bb in range(B):
        m = dict(base_map)
        m["XBF"] = np.ascontiguousarray(xbf[bb])
        in_maps.append(m)

    res = run_bass_kernel_spmd(nc, in_maps, core_ids=list(range(B)))
    out = np.stack(
        [
            np.asarray(r["OUTBF"], ml_dtypes.bfloat16)
            .astype(np.float32)
            .reshape(H, W, C)
            for r in res.results
        ],
        axis=0,
    )
    return out


# revision 17
# speedup vs baseline: 1.0026x; 1.0026x over previous
"""Trainium2 Bass kernel for a neural-cellular-automata step.

out = clip(x + MLP(depthwise3x3(x)), 0, 1) on [8, 512, 512, 16] fp32, NHWC.

Strategy: data-parallel over batch (1 image per NeuronCore, 8 cores).
x is pre-cast to bf16 on the host (tolerance is 2e-2; bf16 keeps us near
4e-3) which halves DMA traffic and makes every PE op run at 1 cycle/row.

Per core the image is processed in 86 column groups: each group's tile
covers 8 w-columns (128 partitions = 8w x 16c) but only the 6 interior
columns are produced, so the 3x3 depthwise taps never cross tiles and no
edge-correction matmuls are needed (tiles overlap by 2 w; group stride 6).
A PE transpose (vs a bf16 identity) turns the native [rows, (w,c)] layout
into X_T [(w,c), rows]; the depthwise perception conv fused with W1
becomes 3 row-shifted bf16 matmuls with block-structured stationary
operands built host-side.  ReLU+bias runs on the scalar engine (bf16
out), the second 1x1 conv uses h as the stationary operand so dx lands
directly back in native layout in PSUM, accumulated over 30-w-column
spans.  The vector engine applies x + dx and the [0,1] clamp into a bf16
staging tile that is DMA-stored incrementally; input DMA is chunked so
compute starts after ~3% of the load.
"""

import contextlib

import numpy as np
import ml_dtypes

import concourse.mybir as mybir
import concourse.tile as tile
from concourse import bacc
from concourse.bass_utils import run_bass_kernel_spmd

B, H, W, C, K, HID = 8, 512, 512, 16, 3, 128
F32 = mybir.dt.float32
F32R = mybir.dt.float32r
BF16 = mybir.dt.bfloat16
GW = 6  # output columns per group (tile covers 8, halo 1 each side)
NG = 86  # 85 full groups + tail group with 2 outputs
PAD_L = 16  # one zero w-column (16 ch) left pad, in elements
ABW = PAD_L + W * C + 80  # band width; right pad so group 85's tile fits
SPAN_W = 30  # w-columns per PSUM dx span (480 native cols)
NBAND = H // 128  # 4 row bands

_CACHE = {}


def _default_perc():
    ident = np.array([[0, 0, 0], [0, 1, 0], [0, 0, 0]], np.float32)
    sx = np.array([[-1, 0, 1], [-2, 0, 2], [-1, 0, 1]], np.float32) / 8.0
    sy = sx.T
    return np.stack([ident, sx, sy], axis=-1)  # [3,3,K]


def _build_lm(perc, W1):
    """Stationary operands for the fused perception+W1 matmuls.

    Block t = (wsl-1)*3 + dy is [128, HID]: rows [wl*16, wl*16+16) hold
    T[dy, 1+(wl-wsl)] for |wl-wsl| <= 1, where wsl in 1..6 is the output
    column within the 8-wide tile and wl the tile-local input column.
    """
    Wk = W1.reshape(C, K, HID)
    T = np.einsum("yxk,ckd->yxcd", perc, Wk).astype(np.float32)  # [3,3,C,HID]
    LM = np.zeros((6, 3, 128, HID), np.float32)
    for wsl in range(1, 7):
        for dy in range(3):
            for dxx in (-1, 0, 1):
                wl = wsl + dxx
                LM[wsl - 1, dy, wl * 16 : wl * 16 + 16, :] = T[dy, 1 + dxx]
    lm = np.ascontiguousarray(LM.transpose(2, 0, 1, 3).reshape(128, 18 * HID))
    return lm.astype(ml_dtypes.bfloat16)


def _build_module():
    nc = bacc.Bacc("TRN2", target_bir_lowering=False)
    XBF = nc.dram_tensor("XBF", [H, W * C], BF16, kind="ExternalInput")
    LM = nc.dram_tensor("LM", [128, 18 * HID], BF16, kind="ExternalInput")
    W2T = nc.dram_tensor("W2T", [HID, C], BF16, kind="ExternalInput")
    B1 = nc.dram_tensor("B1", [HID, 1], F32, kind="ExternalInput")
    IDENT = nc.dram_tensor("IDENT", [128, 128], BF16, kind="ExternalInput")
    OUTBF = nc.dram_tensor("OUTBF", [H, W * C], BF16, kind="ExternalOutput")

    with tile.TileContext(nc) as tc, contextlib.ExitStack() as ctx:
        consts = ctx.enter_context(tc.tile_pool(name="consts", bufs=1))
        hpool = ctx.enter_context(tc.tile_pool(name="hsb", bufs=14))
        otpool = ctx.enter_context(tc.tile_pool(name="ot", bufs=8))
        ps_tr = ctx.enter_context(tc.tile_pool(name="ps_tr", bufs=1, space="PSUM"))
        ps_h = ctx.enter_context(tc.tile_pool(name="ps_h", bufs=5, space="PSUM"))
        ps_dxt = ctx.enter_context(tc.tile_pool(name="ps_dxt", bufs=1, space="PSUM"))

        # queue order matters at startup: ident + the first small x chunks
        # unblock the first transposes; the larger lm load only gates mains.
        ident = consts.tile([128, 128], BF16)
        nc.sync.dma_start(ident, IDENT[:])
        w2 = consts.tile([HID, C], BF16)
        nc.sync.dma_start(w2, W2T[:])
        b1 = consts.tile([HID, 1], F32)
        nc.sync.dma_start(b1, B1[:])

        ab = []
        for b in range(NBAND):
            t = consts.tile([128, ABW], BF16, tag=f"a{b}", name=f"a{b}")
            nc.gpsimd.memset(t[:, 0:PAD_L], 0.0)
            nc.gpsimd.memset(t[:, PAD_L + W * C :], 0.0)
            ab.append(t)
        for b in range(NBAND):
            nc.sync.dma_start(
                ab[b][:, PAD_L : PAD_L + 256],
                XBF[:][128 * b : 128 * b + 128, 0:256],
            )
        lm = consts.tile([128, 18 * HID], BF16)
        nc.sync.dma_start(lm, LM[:])
        for b in range(NBAND):
            nc.sync.dma_start(
                ab[b][:, PAD_L + 256 : PAD_L + 1024],
                XBF[:][128 * b : 128 * b + 128, 256:1024],
            )
        # remaining chunked loads, interleaved across bands
        for c in range(1, 8):
            for b in range(NBAND):
                nc.sync.dma_start(
                    ab[b][:, PAD_L + 1024 * c : PAD_L + 1024 * (c + 1)],
                    XBF[:][128 * b : 128 * b + 128, 1024 * c : 1024 * (c + 1)],
                )

        xts = []
        for i in range(2):
            t = consts.tile([128, 514], BF16, tag=f"xt{i}", name=f"xt{i}")
            nc.gpsimd.memset(t[:, 0:1], 0.0)
            nc.gpsimd.memset(t[:, 513:514], 0.0)
            xts.append(t)

        def lm_ap(wsl, dy):
            t = (wsl - 1) * 3 + dy
            return lm[:, t * HID : (t + 1) * HID]

        def build_xt(j):
            """PE-transpose the 8-w-column window of group j into xts[j%2]."""
            xt = xts[j % 2]
            ptr = ps_tr.tile([128, 512], BF16, tag="tr")
            for b in range(NBAND):
                nc.tensor.matmul(
                    ptr[:, 128 * b : 128 * b + 128],
                    ab[b][:, 96 * j : 96 * j + 128],
                    ident,
                    start=True,
                    stop=True,
                    is_transpose=True,
                )
            nc.vector.tensor_copy(xt[:, 1:513], ptr[:])

        build_xt(0)
        # dx spans: 2 groups (12 w) x 2 bands share one PSUM bank:
        # dxt[p] holds bands 2p, 2p+1 at column offset (b%2)*192.
        dxt = None
        pending = None  # (j, h-pairs) whose W2 matmuls are deferred one group

        def emit_w2(jj, hps):
            nonlocal dxt
            if jj % 2 == 0:
                dxt = [
                    ps_dxt.tile([128, 384], F32, tag=f"d{p}", name=f"d{p}")
                    for p in range(2)
                ]
            nout = 2 if jj == NG - 1 else GW
            for wsl in range(1, nout + 1):
                w = 6 * jj - 1 + wsl
                off = (w % 12) * 16
                for b in range(NBAND):
                    nc.tensor.matmul(
                        dxt[b // 2][:, (b % 2) * 192 + off : (b % 2) * 192 + off + 16],
                        hps[wsl - 1][:, 128 * b : 128 * b + 128],
                        w2[:],
                        start=True,
                        stop=True,
                    )

        def emit_add_store(jj):
            """Residual add + clamp + store once group jj completed a span."""
            if jj % 2 != 1 and jj != NG - 1:
                return
            S = jj // 2
            ncb = 192 if S < 42 else 128
            for b in range(NBAND):
                ot = otpool.tile([128, ncb], BF16, tag="ot")
                nc.vector.tensor_tensor(
                    ot,
                    dxt[b // 2][:, (b % 2) * 192 : (b % 2) * 192 + ncb],
                    ab[b][:, PAD_L + 192 * S : PAD_L + 192 * S + ncb],
                    op=mybir.AluOpType.add,
                )
                nc.vector.tensor_scalar(
                    ot, ot, 0.0, 1.0, mybir.AluOpType.max, mybir.AluOpType.min
                )
                nc.sync.dma_start(
                    OUTBF[:][128 * b : 128 * b + 128, 192 * S : 192 * S + ncb],
                    ot,
                )

        for j in range(NG):
            if j + 1 < NG:
                build_xt(j + 1)
            xt = xts[j % 2]

            nout = 2 if j == NG - 1 else GW
            hps = []
            for wsl in range(1, nout + 1):
                hp = ps_h.tile([128, 512], F32, tag="h")
                for dy in range(3):
                    nc.tensor.matmul(
                        hp,
                        lm_ap(wsl, dy),
                        xt[:, dy : dy + 512],
                        start=(dy == 0),
                        stop=(dy == 2),
                    )
                h = hpool.tile([HID, 512], BF16, tag="h")
                nc.scalar.activation(
                    h, hp, mybir.ActivationFunctionType.Relu, bias=b1, scale=1.0
                )
                hps.append(h)

            if pending is not None:
                jj, phps = pending
                emit_w2(jj, phps)
                emit_add_store(jj)
            pending = (j, hps)

        jj, phps = pending
        emit_w2(jj, phps)
        emit_add_store(jj)

    nc.finalize()
    return nc


def kernel(x, perc=None, W1=None, b1=None, W2=None, b2=None, lock_release=None, **_):
    x = np.asarray(x)
    assert x.shape == (B, H, W, C)
    x = np.ascontiguousarray(x, np.float32)
    perc = _default_perc() if perc is None else np.asarray(perc, np.float32)
    W1 = np.asarray(W1, np.float32)
    W2 = np.asarray(W2, np.float32)
    b1 = np.zeros(HID, np.float32) if b1 is None else np.asarray(b1, np.float32)
    b2 = np.zeros(C, np.float32) if b2 is None else np.asarray(b2, np.float32)
    assert not np.any(b2 != 0.0), "fast path assumes b2 == 0"

    key = ("mod", False)
    if key not in _CACHE:
        _CACHE[key] = _build_module()
    nc = _CACHE[key]

    base_map = {
        "LM": _build_lm(perc, W1),
        "W2T": np.ascontiguousarray(W2).astype(ml_dtypes.bfloat16),
        "B1": np.ascontiguousarray(b1[:, None]),
        "IDENT": np.eye(128, dtype=np.float32).astype(ml_dtypes.bfloat16),
    }
    xbf = x.reshape(B, H, W * C).astype(ml_dtypes.bfloat16)

    in_maps = []
    for bb in range(B):
        m = dict(base_map)
        m["XBF"] = np.ascontiguousarray(xbf[bb])
        in_maps.append(m)

    res = run_bass_kernel_spmd(nc, in_maps, core_ids=list(range(B)))
    out = np.stack(
        [
            np.asarray(r["OUTBF"], ml_dtypes.bfloat16)
            .astype(np.float32)
            .reshape(H, W, C)
            for r in res.results
        ],
        axis=0,
    )
    return out
